# revision 26
# baseline (speedup 1.0000x reference)
# Trainium2 Bass kernel for nn_MCorrLCorr (Mellin-correlation along x,
# linear correlation along y).
#
#   out[b,o,hx,hy] = bias[o]
#     + sum_{c,fx,fy} input[b, c, (hx+1)*(fx+1)-1, 2*hy + fy - 2] * weight[o,c,fx,fy]
#   (terms with 2*hy+fy-2 < 0 dropped; only hy=0, fy<2)
#
# Per core (2 batches, data-parallel over 8 cores), pipelined in 16-hx chunks:
#   1. x-gather: 4 strided DMAs per chunk (one per fx) load
#      S[(fx,c)=128, l=16, gy=384] fp32 from HBM. The three DMA rings carry
#      no gated compute so prefetch flows freely: sync ring fx3, scalar
#      ring fx0+fx2, gpsimd ring fx1 + outputs (balanced by the HBM
#      stride penalty fx+1).
#   2. cast + parity split on DVE: even/odd gy copied with fp32 -> bf16
#      cast into Xe/Xo[(fx,c), l, 194] so every matmul's moving operand is
#      CONTIGUOUS bf16. Index 0 / 193 are zeros (dropped y terms).
#   3. matmul: same-parity fy pairs (fy, fy+2) share one moving stream
#      shifted by one hy. Stationary [W_fy | W_fy+2] (K=128 x M=128), one
#      matmul over X?[:, l0:l0+2, off:off+192] (N=384) computes both fy:
#      PSUM rows 0:64 = fy_lo sums at hy=n, rows 64:128 = fy_hi at hy=n-1.
#      Bias is folded in as a 5th accumulating matmul (stationary has
#      bias[o] on partition 0 / cols 0:64, moving is a ones tile).
#      4+1 matmuls accumulate per bank; each stationary sweeps 8 banks.
#   4. combine on DVE at full lane width: one 128-partition copy per PSUM
#      tile into an SBUF staging tile, then a single shifted add
#      out[o] = stage[o, hy] + stage[64+o, hy+1] per chunk; ONE output DMA
#      per chunk (64 x 12 KB contiguous descriptors).

import ml_dtypes
import numpy as np

import concourse.bass as bass
import concourse.mybir as mybir
import concourse.tile as tile
from concourse import bacc
from concourse.bass_utils import run_bass_kernel_spmd

B, C, NGX, NGY = 16, 32, 128, 384
O, NFX, NFY = 64, 4, 8
NHX, NHY = 32, 190
NCORES = 8
BPC = B // NCORES  # batches per core
F32 = mybir.dt.float32
BF16 = mybir.dt.bfloat16

HX_TILE = 2  # output hx rows per PSUM bank slot
NMM = NHY + 2  # moving columns per matmul per hx row
NPAR = NHY + 4  # parity-tile columns: [zero, 192 gy values, zero]
PAIR_LO = (0, 1, 4, 5)  # fy pairs (lo, lo+2)
NSLOT = len(PAIR_LO)  # 4 fy pairs
NGRP = 8  # PSUM bank slots swept per stationary load
GT = NGRP // 2  # bank slots per PSUM tile (4 banks)
HCH = NGRP * HX_TILE  # hx rows per chunk (16)
NCHUNK = NHX // HCH  # chunks per batch (2)


def build_nc():
    nc = bacc.Bacc("TRN2", target_bir_lowering=False)
    inp = nc.dram_tensor("input", [BPC, C, NGX, NGY], F32, kind="ExternalInput")
    wre = nc.dram_tensor("weight", [NFX * C, NSLOT, 128], BF16, kind="ExternalInput")
    bia = nc.dram_tensor("bias", [O, 1], F32, kind="ExternalInput")
    out = nc.dram_tensor("out", [BPC, O, NHX, NHY], F32, kind="ExternalOutput")
    inp_ap, wre_ap, bia_ap, out_ap = inp.ap(), wre.ap(), bia.ap(), out.ap()

    with tile.TileContext(nc) as tc:
        with (
            tc.tile_pool(name="consts", bufs=1) as consts,
            tc.tile_pool(name="xst", bufs=4) as stpool,
            tc.tile_pool(name="xpar", bufs=3) as parpool,
            tc.tile_pool(name="obc", bufs=2) as opool,
            tc.tile_pool(name="ps", bufs=2, space="PSUM") as pspool,
        ):
            w_sb = consts.tile([NFX * C, NSLOT, 128], BF16)
            nc.sync.dma_start(out=w_sb, in_=wre_ap)
            bias_sb = consts.tile([O, 1], F32)
            nc.sync.dma_start(out=bias_sb, in_=bia_ap)

            for b in range(BPC):
                for ch in range(NCHUNK):
                    hxb = ch * HCH  # first global hx row of this chunk
                    # S[(fx,c), l, gy] = input[b, c, (hxb+l+1)*(fx+1)-1, gy]
                    xst = stpool.tile(
                        [NFX * C, HCH, NGY], F32, tag="xst", name=f"xst_{b}_{ch}"
                    )
                    for fx in range(NFX):
                        row0 = (hxb + 1) * (fx + 1) - 1
                        src = bass.AP(
                            inp_ap.tensor,
                            b * C * NGX * NGY + row0 * NGY,
                            [[NGX * NGY, C], [(fx + 1) * NGY, HCH], [1, NGY]],
                        )
                        dst = xst[fx * C : (fx + 1) * C, :, :]
                        if fx == 3:
                            nc.sync.dma_start(out=dst, in_=src)
                        elif fx == 1:
                            nc.gpsimd.dma_start(out=dst, in_=src)
                        else:
                            nc.scalar.dma_start(out=dst, in_=src)

                    # parity split + cast: X[q][p, l, 1+k] = S[p, l, 2k+q]
                    xe = parpool.tile(
                        [NFX * C, HCH, NPAR], BF16, tag="xe", name=f"xe_{b}_{ch}"
                    )
                    xo = parpool.tile(
                        [NFX * C, HCH, NPAR], BF16, tag="xo", name=f"xo_{b}_{ch}"
                    )
                    nc.vector.memset(xe[:, :, 0:1], 0.0)
                    nc.vector.memset(xe[:, :, NPAR - 1 : NPAR], 0.0)
                    nc.vector.memset(xo[:, :, 0:1], 0.0)
                    nc.vector.memset(xo[:, :, NPAR - 1 : NPAR], 0.0)
                    nc.vector.tensor_copy(xe[:, :, 1 : NPAR - 1], xst[:, :, 0:NGY:2])
                    nc.vector.tensor_copy(xo[:, :, 1 : NPAR - 1], xst[:, :, 1:NGY:2])
                    xq = (xe, xo)

                    # two 4-bank PSUM tiles per chunk; bank slot = 512 fp32
                    pst = [
                        pspool.tile(
                            [128, GT, 512], F32, tag="ps", name=f"ps_{b}_{ch}_{t}"
                        )
                        for t in range(2)
                    ]
                    for pr in range(NSLOT):
                        for g in range(NGRP):
                            t, gl = divmod(g, GT)
                            l0 = g * HX_TILE
                            fy_lo = PAIR_LO[pr]
                            q, off = fy_lo & 1, (fy_lo - (fy_lo & 1)) // 2
                            rhs = xq[q][:, l0 : l0 + HX_TILE, off : off + NMM]
                            out_ps = pst[t][:, gl, 0 : 2 * NMM].rearrange(
                                "p (a b) -> p a b", a=HX_TILE
                            )
                            nc.tensor.matmul(
                                out_ps,
                                w_sb[:, pr, :],
                                rhs,
                                start=(pr == 0),
                                stop=(pr == NSLOT - 1),
                            )

                    obc = opool.tile(
                        [O, HCH, NHY], F32, tag="obc", name=f"obc_{b}_{ch}"
                    )
                    for g in range(NGRP):
                        t, gl = divmod(g, GT)
                        l0 = g * HX_TILE
                        ps = pst[t][:, gl, 0 : 2 * NMM].rearrange(
                            "p (a b) -> p a b", a=HX_TILE
                        )
                        ob = obc[:, l0 : l0 + HX_TILE, :]
                        # rows 0:64: fy_lo sums at hy=n; add bias while copying
                        nc.scalar.add(ob, ps[0:O, :, 0:NHY], bias_sb)
                        # rows 64:128: fy_hi sums at hy=n-1 -> shift left by one
                        nc.vector.tensor_add(ob, ob, ps[O:128, :, 1 : NHY + 1])
                    nc.gpsimd.dma_start(
                        out=out_ap[b, :, hxb : hxb + HCH, :], in_=obc
                    )
    nc.compile()
    return nc


def _prep_maps(inputs):
    inp = np.ascontiguousarray(np.asarray(inputs["input"], dtype=np.float32))
    w = np.asarray(inputs["weight"], dtype=np.float32)
    bias = np.asarray(inputs["bias"], dtype=np.float32)
    # wt[fx*C + c, fy, o] = weight[o, c, fx, fy]
    wt = w.transpose(2, 1, 3, 0).reshape(NFX * C, NFY, O)
    w2 = np.zeros((NFX * C, NSLOT, 128), np.float32)
    for pr, fy_lo in enumerate(PAIR_LO):
        w2[:, pr, 0:O] = wt[:, fy_lo]
        w2[:, pr, O:128] = wt[:, fy_lo + 2]
    w2 = np.ascontiguousarray(w2.astype(ml_dtypes.bfloat16))
    bre = np.ascontiguousarray(bias.reshape(O, 1))
    return [
        {
            "input": np.ascontiguousarray(inp[k * BPC : (k + 1) * BPC]),
            "weight": w2,
            "bias": bre,
        }
        for k in range(NCORES)
    ]


def kernel(**inputs) -> np.ndarray:
    nc = build_nc()
    in_maps = _prep_maps(inputs)
    res = run_bass_kernel_spmd(nc, in_maps, core_ids=list(range(NCORES)))
    return np.concatenate([r["out"] for r in res.results], axis=0)


# revision 27
# speedup vs baseline: 1.1175x; 1.1175x over previous
# Trainium2 Bass kernel for nn_MCorrLCorr (Mellin-correlation along x,
# linear correlation along y).
#
#   out[b,o,hx,hy] = bias[o]
#     + sum_{c,fx,fy} input[b, c, (hx+1)*(fx+1)-1, 2*hy + fy - 2] * weight[o,c,fx,fy]
#   (terms with 2*hy+fy-2 < 0 dropped; only hy=0, fy<2)
#
# Per core (2 batches, data-parallel over 8 cores), pipelined in 16-hx chunks:
#   1. x-gather: 4 strided DMAs per chunk (one per fx) load
#      S[(fx,c)=128, l=16, gy=384] fp32 from HBM. The three DMA rings carry
#      no gated compute so prefetch flows freely: sync ring fx3, scalar
#      ring fx0+fx2, gpsimd ring fx1 + outputs (balanced by the HBM
#      stride penalty fx+1).
#   2. cast + parity split on DVE: even/odd gy copied with fp32 -> bf16
#      cast into Xe/Xo[(fx,c), l, 194] so every matmul's moving operand is
#      CONTIGUOUS bf16. Index 0 / 193 are zeros (dropped y terms).
#   3. matmul: same-parity fy pairs (fy, fy+2) share one moving stream
#      shifted by one hy. Stationary [W_fy | W_fy+2] (K=128 x M=128), one
#      matmul over X?[:, l0:l0+2, off:off+192] (N=384) computes both fy:
#      PSUM rows 0:64 = fy_lo sums at hy=n, rows 64:128 = fy_hi at hy=n-1.
#      Bias is folded in as a 5th accumulating matmul (stationary has
#      bias[o] on partition 0 / cols 0:64, moving is a ones tile).
#      4+1 matmuls accumulate per bank; each stationary sweeps 8 banks.
#   4. combine on DVE at full lane width: one 128-partition copy per PSUM
#      tile into an SBUF staging tile, then a single shifted add
#      out[o] = stage[o, hy] + stage[64+o, hy+1] per chunk; ONE output DMA
#      per chunk (64 x 12 KB contiguous descriptors).

import ml_dtypes
import numpy as np

import concourse.bass as bass
import concourse.mybir as mybir
import concourse.tile as tile
from concourse import bacc
from concourse.bass_utils import run_bass_kernel_spmd

B, C, NGX, NGY = 16, 32, 128, 384
O, NFX, NFY = 64, 4, 8
NHX, NHY = 32, 190
NCORES = 8
BPC = B // NCORES  # batches per core
F32 = mybir.dt.float32
BF16 = mybir.dt.bfloat16

HX_TILE = 2  # output hx rows per PSUM bank slot
NMM = NHY + 2  # moving columns per matmul per hx row
NPAR = NHY + 4  # parity-tile columns: [zero, 192 gy values, zero]
PAIR_LO = (0, 1, 4, 5)  # fy pairs (lo, lo+2)
NSLOT = len(PAIR_LO)  # 4 fy pairs
NGRP = 8  # PSUM bank slots swept per stationary load
GT = NGRP // 2  # bank slots per PSUM tile (4 banks)
HCH = NGRP * HX_TILE  # hx rows per chunk (16)
NCHUNK = NHX // HCH  # chunks per batch (2)


def build_nc():
    nc = bacc.Bacc("TRN2", target_bir_lowering=False)
    inp = nc.dram_tensor("input", [BPC, C, NGX, NGY], F32, kind="ExternalInput")
    wre = nc.dram_tensor("weight", [NFX * C, NSLOT, 128], BF16, kind="ExternalInput")
    bia = nc.dram_tensor("bias", [O, 1], F32, kind="ExternalInput")
    out = nc.dram_tensor("out", [BPC, O, NHX, NHY], F32, kind="ExternalOutput")
    inp_ap, wre_ap, bia_ap, out_ap = inp.ap(), wre.ap(), bia.ap(), out.ap()

    with tile.TileContext(nc) as tc:
        with (
            tc.tile_pool(name="consts", bufs=1) as consts,
            tc.tile_pool(name="xst", bufs=4) as stpool,
            tc.tile_pool(name="xpar", bufs=3) as parpool,
            tc.tile_pool(name="obc", bufs=2) as opool,
            tc.tile_pool(name="ps", bufs=8, space="PSUM") as pspool,
        ):
            w_sb = consts.tile([NFX * C, NSLOT, 128], BF16)
            nc.sync.dma_start(out=w_sb, in_=wre_ap)
            bias_sb = consts.tile([O, 1], F32)
            nc.sync.dma_start(out=bias_sb, in_=bia_ap)

            for b in range(BPC):
                for ch in range(NCHUNK):
                    hxb = ch * HCH  # first global hx row of this chunk
                    # S[(fx,c), l, gy] = input[b, c, (hxb+l+1)*(fx+1)-1, gy]
                    xst = stpool.tile(
                        [NFX * C, HCH, NGY], F32, tag="xst", name=f"xst_{b}_{ch}"
                    )
                    for fx in range(NFX):
                        row0 = (hxb + 1) * (fx + 1) - 1
                        src = bass.AP(
                            inp_ap.tensor,
                            b * C * NGX * NGY + row0 * NGY,
                            [[NGX * NGY, C], [(fx + 1) * NGY, HCH], [1, NGY]],
                        )
                        dst = xst[fx * C : (fx + 1) * C, :, :]
                        if fx == 3:
                            nc.sync.dma_start(out=dst, in_=src)
                        elif fx == 2:
                            nc.gpsimd.dma_start(out=dst, in_=src)
                        else:
                            nc.scalar.dma_start(out=dst, in_=src)

                    # parity split + cast: X[q][p, l, 1+k] = S[p, l, 2k+q]
                    xe = parpool.tile(
                        [NFX * C, HCH, NPAR], BF16, tag="xe", name=f"xe_{b}_{ch}"
                    )
                    xo = parpool.tile(
                        [NFX * C, HCH, NPAR], BF16, tag="xo", name=f"xo_{b}_{ch}"
                    )
                    nc.vector.memset(xe[:, :, 0:1], 0.0)
                    nc.vector.memset(xe[:, :, NPAR - 1 : NPAR], 0.0)
                    nc.vector.memset(xo[:, :, 0:1], 0.0)
                    nc.vector.memset(xo[:, :, NPAR - 1 : NPAR], 0.0)
                    nc.vector.tensor_copy(xe[:, :, 1 : NPAR - 1], xst[:, :, 0:NGY:2])
                    nc.scalar.copy(xo[:, :, 1 : NPAR - 1], xst[:, :, 1:NGY:2])
                    xq = (xe, xo)

                    pss = [
                        pspool.tile(
                            [128, HX_TILE, NMM], F32, tag="ps", name=f"ps_{b}_{ch}_{g}"
                        )
                        for g in range(NGRP)
                    ]
                    for pr in range(NSLOT):
                        for g in range(NGRP):
                            l0 = g * HX_TILE
                            fy_lo = PAIR_LO[pr]
                            q, off = fy_lo & 1, (fy_lo - (fy_lo & 1)) // 2
                            rhs = xq[q][:, l0 : l0 + HX_TILE, off : off + NMM]
                            nc.tensor.matmul(
                                pss[g],
                                w_sb[:, pr, :],
                                rhs,
                                start=(pr == 0),
                                stop=(pr == NSLOT - 1),
                            )

                    obc = opool.tile(
                        [O, HCH, NHY], F32, tag="obc", name=f"obc_{b}_{ch}"
                    )
                    for g in range(NGRP):
                        l0 = g * HX_TILE
                        ps = pss[g]
                        ob = obc[:, l0 : l0 + HX_TILE, :]
                        # rows 0:64: fy_lo sums at hy=n; add bias while copying
                        nc.scalar.add(ob, ps[0:O, :, 0:NHY], bias_sb)
                        # rows 64:128: fy_hi sums at hy=n-1 -> shift left by one
                        nc.vector.tensor_add(ob, ob, ps[O:128, :, 1 : NHY + 1])
                    nc.gpsimd.dma_start(
                        out=out_ap[b, :, hxb : hxb + HCH, :], in_=obc
                    )
    nc.compile()
    return nc


def _prep_maps(inputs):
    inp = np.ascontiguousarray(np.asarray(inputs["input"], dtype=np.float32))
    w = np.asarray(inputs["weight"], dtype=np.float32)
    bias = np.asarray(inputs["bias"], dtype=np.float32)
    # wt[fx*C + c, fy, o] = weight[o, c, fx, fy]
    wt = w.transpose(2, 1, 3, 0).reshape(NFX * C, NFY, O)
    w2 = np.zeros((NFX * C, NSLOT, 128), np.float32)
    for pr, fy_lo in enumerate(PAIR_LO):
        w2[:, pr, 0:O] = wt[:, fy_lo]
        w2[:, pr, O:128] = wt[:, fy_lo + 2]
    w2 = np.ascontiguousarray(w2.astype(ml_dtypes.bfloat16))
    bre = np.ascontiguousarray(bias.reshape(O, 1))
    return [
        {
            "input": np.ascontiguousarray(inp[k * BPC : (k + 1) * BPC]),
            "weight": w2,
            "bias": bre,
        }
        for k in range(NCORES)
    ]


def kernel(**inputs) -> np.ndarray:
    nc = build_nc()
    in_maps = _prep_maps(inputs)
    res = run_bass_kernel_spmd(nc, in_maps, core_ids=list(range(NCORES)))
    return np.concatenate([r["out"] for r in res.results], axis=0)
